# revision 6
# baseline (speedup 1.0000x reference)
"""Multi-head attention (B=2, S=2048, D=1024, H=16) on 8 trn2 NeuronCores.

Sharding: core c -> batch b = c // 4, head group g = c % 4 (heads 4g..4g+3).
Each core computes, for its batch shard and 4 heads:
  QT/KT = (x W + b)^T in [d_local, seq] layout, V in [seq, d_local] layout,
  transposed scores S^T[k, q] = K Q^T (so softmax needs no transposes),
  exp via ACT (scale fused), fp8e4m3 DoubleRow PV matmul with an appended
  ones column which yields both the unnormalized context and the softmax
  row sums, normalization via a gpsimd partition-broadcast reciprocal
  multiply, and a partial output projection against a row shard of Wo.
Host sums the 4 partials per batch and adds the constant row bv @ Wo + bo
(softmax rows sum to one, so bv's contribution is a constant vector).

The schedule is paced by the ACT engine (exp of 16.8M scores = ~110us of
ACT work, the largest single-engine floor):
  - x arrives in per-chunk token-halves; Q/K dblk0 projections consume
    each half-chunk on arrival so the first exp starts ~20us in.
  - scores are emitted as (key-chunk, token-half) units of [128,1024],
    each followed by its exp; PE fillers (V projection, dblk1/second-half
    projections, fp8 PV pairs) are spread per-unit so the PE never runs
    far ahead or behind the ACT cadence.
  - long-lived projection accumulators live in the ctx PSUM banks between
    heads (the 2-buffer work pool would deadlock on them).
  - tail: head-3 normalize in 4 parts; out-projection PSUM->SBUF movers
    alternate between DVE and ACT (idle after its last exp); output DMA
    alternates between the sync and gpsimd queues.
"""

import sys

sys.path.insert(0, "/opt/trn_rl_repo")

import numpy as np
import ml_dtypes

B = 2
S = 2048
D = 1024
H = 16
HD = 64
NCORES = 8
HPC = 4          # heads per core
DL = HPC * HD    # 256 local head dims per core
P = 128
KCH = S // P     # 16 key chunks
DCH = D // P     # 8 contraction chunks
TBLK = S // P    # 16 token blocks
NPAIR = KCH // 2
SCALE = 1.0 / np.sqrt(HD)

# fp8e4m3 + DoubleRow PV halves PV matmul time at ~1.1e-2 extra relative
# error (fp8 noise on the softmax probabilities does not average away
# relative to the softmax-averaged context). Total stays under the 2e-2
# gate, and the freed PE time lets the exp (ACT) engine pace the kernel.
USE_FP8_PV = True

_CACHE = {}


def _build():
    import concourse.bass as bass  # noqa: F401
    import concourse.mybir as mybir
    import concourse.tile as tile
    from concourse import bacc

    bf16 = mybir.dt.bfloat16
    f32 = mybir.dt.float32
    fp8 = mybir.dt.float8e4
    DR = mybir.MatmulPerfMode.DoubleRow
    Exp = mybir.ActivationFunctionType.Exp

    nc = bacc.Bacc("TRN2", target_bir_lowering=False, debug=False,
                   num_devices=NCORES)

    xT_d = nc.dram_tensor("xt", [D, S], bf16, kind="ExternalInput")
    wq_d = nc.dram_tensor("wq", [D, DL], bf16, kind="ExternalInput")
    wk_d = nc.dram_tensor("wk", [D, DL], bf16, kind="ExternalInput")
    wv_d = nc.dram_tensor("wv", [D, DL], bf16, kind="ExternalInput")
    wo_d = nc.dram_tensor("wo", [DL, D], bf16, kind="ExternalInput")
    bqk_d = nc.dram_tensor("bqk", [P, 4], f32, kind="ExternalInput")
    out_d = nc.dram_tensor("out", [S, D], bf16, kind="ExternalOutput")

    att_dt = fp8 if USE_FP8_PV else bf16

    with tile.TileContext(nc) as tc:
        with (
            tc.tile_pool(name="persist", bufs=1) as pp,
            tc.tile_pool(name="stream", bufs=3) as sp,
            tc.tile_pool(name="psum", bufs=2, space="PSUM") as ps,
        ):
            # ---- input DMAs: weights first (small), then x token-halves.
            # sync queue : bqk, wk, x-h0 even chunks, x-h1 odd chunks
            # gpsimd q   : wq, x-h0 odd chunks, wv, x-h1 even chunks, wo
            bqk_s = pp.tile([P, 4], f32, tag="bqk", name="bqk_s")
            nc.sync.dma_start(bqk_s[:], bqk_d[:])
            wq_s = pp.tile([P, DCH, DL], bf16, tag="wq", name="wq_s")
            wk_s = pp.tile([P, DCH, DL], bf16, tag="wk", name="wk_s")
            wv_s = pp.tile([P, DCH, DL], bf16, tag="wv", name="wv_s")
            xts = [pp.tile([P, S], bf16, tag=f"xt{c}", name=f"xt{c}")
                   for c in range(DCH)]
            for c in range(DCH):
                nc.gpsimd.dma_start(wq_s[:, c, :], wq_d[c * P:(c + 1) * P, :])
            for c in range(DCH):
                nc.sync.dma_start(wk_s[:, c, :], wk_d[c * P:(c + 1) * P, :])
            # x first token-halves, alternating queues by chunk parity
            for c in range(DCH):
                eng = nc.sync if c % 2 == 0 else nc.gpsimd
                eng.dma_start(xts[c][:, 0:1024], xT_d[c * P:(c + 1) * P, 0:1024])
            for c in range(DCH):
                nc.gpsimd.dma_start(wv_s[:, c, :], wv_d[c * P:(c + 1) * P, :])
            # x second token-halves
            for c in range(DCH):
                eng = nc.sync if c % 2 == 1 else nc.gpsimd
                eng.dma_start(xts[c][:, 1024:2048],
                              xT_d[c * P:(c + 1) * P, 1024:2048])
            wo_s = pp.tile([P, 2, D], bf16, tag="wo", name="wo_s")
            for dc in range(2):
                nc.gpsimd.dma_start(wo_s[:, dc, :], wo_d[dc * P:(dc + 1) * P, :])

            qt = [pp.tile([P, S], bf16, tag=f"qt{d}", name=f"qt{d}")
                  for d in range(2)]
            kt = [pp.tile([P, S], bf16, tag=f"kt{d}", name=f"kt{d}")
                  for d in range(2)]

            # ---- phase B: Q/K dblk0 first-token-half projections,
            # consuming each x half-chunk once on arrival.
            qacc = ps.tile([P, 1024], f32, tag="work", name="qacc_h0")
            kacc = ps.tile([P, 1024], f32, tag="work", name="kacc_h0")
            for c in range(DCH):
                for ns in range(2):
                    nc.tensor.matmul(
                        qacc[:, ns * 512:(ns + 1) * 512],
                        wq_s[:, c, 0:P],
                        xts[c][:, ns * 512:(ns + 1) * 512],
                        start=(c == 0), stop=(c == DCH - 1))
                for ns in range(2):
                    nc.tensor.matmul(
                        kacc[:, ns * 512:(ns + 1) * 512],
                        wk_s[:, c, 0:P],
                        xts[c][:, ns * 512:(ns + 1) * 512],
                        start=(c == 0), stop=(c == DCH - 1))
            nc.vector.tensor_scalar_add(qt[0][:, 0:1024], qacc[:],
                                        bqk_s[:, 0:1])
            nc.vector.tensor_scalar_add(kt[0][:, 0:1024], kacc[:],
                                        bqk_s[:, 2:3])

            # V tiles: k-chunk PAIRS [128, 2, 4 heads * 68]; col 68h+64 is
            # the softmax-sum ones column (65..67 pad for 16B alignment).
            vts = [None] * NPAIR

            def v_proj(tb):
                pr, j = tb // 2, tb % 2
                if j == 0:
                    vt = pp.tile([P, 2, HPC * 68], att_dt, tag=f"v{pr}",
                                 name=f"v{pr}")
                    v4 = vt.rearrange("p j (h e) -> p j h e", e=68)
                    nc.gpsimd.memset(v4[:, :, :, 64:65], 1.0)
                    vts[pr] = vt
                vt = vts[pr]
                v4 = vt.rearrange("p j (h e) -> p j h e", e=68)
                acc = ps.tile([P, 1024], f32, tag="work", name=f"ps_v{tb}")
                for kc in range(DCH):
                    nc.tensor.matmul(
                        acc[:, 0:DL],
                        xts[kc][:, tb * P:(tb + 1) * P],
                        wv_s[:, kc, :],
                        start=(kc == 0), stop=(kc == DCH - 1))
                nc.vector.tensor_copy(
                    v4[:, j, :, 0:64],
                    acc[:, 0:DL].rearrange("p (h e) -> p h e", e=64))

            # Long-lived projection accumulators share the ctx PSUM banks:
            # one [P,2048] tile holds a Q-half acc and a K-half acc side by
            # side; its MMs interleave freely with the scores work pool.
            def proj_acc_alloc(name):
                return ps.tile([P, S], f32, tag="ctx", bufs=1, name=name)

            def proj_mm(accqk, which, dblk, half, c):
                """2 MMs of chunk c into acc half (which: 0=Q, 1=K)."""
                w_s = wq_s if which == 0 else wk_s
                base = 1024 * which
                for ns in range(2):
                    nc.tensor.matmul(
                        accqk[:, base + ns * 512:base + (ns + 1) * 512],
                        w_s[:, c, dblk * P:(dblk + 1) * P],
                        xts[c][:, half * 1024 + ns * 512:
                               half * 1024 + (ns + 1) * 512],
                        start=(c == 0), stop=(c == DCH - 1))

            def proj_bias(accqk, dblk, half):
                nc.vector.tensor_scalar_add(
                    qt[dblk][:, half * 1024:(half + 1) * 1024],
                    accqk[:, 0:1024], bqk_s[:, dblk:dblk + 1])
                nc.vector.tensor_scalar_add(
                    kt[dblk][:, half * 1024:(half + 1) * 1024],
                    accqk[:, 1024:2048], bqk_s[:, 2 + dblk:3 + dblk])

            etps = [None] * NPAIR
            ctx_ps_ref = [None]
            ctx_sb = [pp.tile([P, S], bf16, tag=f"ctx{dc}", name=f"ctx{dc}")
                      for dc in range(2)]

            def scores_unit(h, kc, qh):
                """[128k x 1024q] scores + exp for one (chunk, token-half)."""
                dblk = h // 2
                roff = 64 * (h % 2)
                pr, j = kc // 2, kc % 2
                if j == 0 and qh == 0:
                    # bufs=8 keeps every pair of a head live: with the
                    # token-half-major unit order, PV trails the allocs by
                    # up to a full head, so 4 rotating buffers would stall
                    # the exp of unit 2*pr on PV(pr-4).
                    etps[pr] = sp.tile([P, 2, S], att_dt, tag="expt", bufs=8,
                                       name=f"expt{h}_{pr}")
                sc = ps.tile([P, 1024], f32, tag="work",
                             name=f"sc{h}_{kc}_{qh}")
                for ns in range(2):
                    nc.tensor.matmul(
                        sc[:, ns * 512:(ns + 1) * 512],
                        kt[dblk][roff:roff + 64, kc * P:(kc + 1) * P],
                        qt[dblk][roff:roff + 64,
                                 qh * 1024 + ns * 512:qh * 1024 + (ns + 1) * 512],
                        start=True, stop=True)
                nc.scalar.activation(
                    etps[pr][:, j, qh * 1024:(qh + 1) * 1024], sc[:],
                    Exp, scale=float(SCALE))

            def pv_pair(h, pr):
                if pr == 0:
                    ctx_ps_ref[0] = ps.tile([P, S], f32, tag="ctx", bufs=1,
                                            name=f"ps_ctx{h}")
                ctx_ps = ctx_ps_ref[0]
                v4 = vts[pr].rearrange("p j (h e) -> p j h e", e=68)
                if USE_FP8_PV:
                    for ns in range(4):
                        nc.tensor.matmul(
                            ctx_ps[0:65, ns * 512:(ns + 1) * 512],
                            v4[:, :, h, 0:65],
                            etps[pr][:, :, ns * 512:(ns + 1) * 512],
                            start=(pr == 0), stop=(pr == NPAIR - 1),
                            perf_mode=DR)
                else:
                    for j in range(2):
                        for ns in range(4):
                            nc.tensor.matmul(
                                ctx_ps[0:65, ns * 512:(ns + 1) * 512],
                                v4[:, j, h, 0:65],
                                etps[pr][:, j, ns * 512:(ns + 1) * 512],
                                start=(pr == 0 and j == 0),
                                stop=(pr == NPAIR - 1 and j == 1))

            def normalize(h, part=0, nparts=2):
                """Normalize one 1/nparts slice of head h's context."""
                dblk = h // 2
                roff = 64 * (h % 2)
                w = S // nparts
                ctx_ps = ctx_ps_ref[0]
                hs = slice(part * w, (part + 1) * w)
                srow = sp.tile([1, w], f32, tag=f"srow{w}", bufs=2,
                               name=f"srow{h}_{part}")
                nc.vector.tensor_copy(srow[:], ctx_ps[64:65, hs])
                rec = sp.tile([1, w], f32, tag=f"rec{w}", bufs=2,
                              name=f"rec{h}_{part}")
                nc.vector.reciprocal_approx_fast(rec[:], srow[:])
                bc = sp.tile([64, w], f32, tag=f"bc{w}", bufs=2,
                             name=f"bc{h}_{part}")
                nc.gpsimd.partition_broadcast(bc[:], rec[:])
                nc.vector.tensor_mul(
                    ctx_sb[dblk][roff:roff + 64, hs],
                    ctx_ps[0:64, hs], bc[:])

            # ---- heads loop: 32 ACT-paced units per head (~1.28us each)
            # with PE fillers budgeted to ~850ns per unit on top of the
            # 427ns scores MMs:
            #   head 0: u0-7 K0h1+Q0h1 proj chunks; u8-21 V tb 0-13;
            #           PV pairs 0-6 late in the head.
            #   head 1: u0-1 V tb 14-15, u2 PV(0,7), u3-4 normalize(0),
            #           u8-15 dblk1-h0 proj acc, u16-23 dblk1-h1 proj acc
            #           (both in the ctx PSUM banks), PV from u24.
            #   heads 2-3: scores + PV only (slack absorbs HAM throttle).
            # Pair pr's exps complete after unit 17+2*pr; each PV emission
            # map respects that with one unit of margin, plus the ctx-bank
            # hand-off order (proj accs -> ctx) within the bufs=1 pool.

            def head_units():
                return ([(kc, 0) for kc in range(KCH)] +
                        [(kc, 1) for kc in range(KCH)])

            def pv_at_map(base):
                m = {}
                for pr in range(NPAIR - 1):
                    m[max(base + pr, 18 + 2 * pr)] = pr
                return m

            # --- head 0 ---
            h0_acc = proj_acc_alloc("accQK0h1")
            pv_at = pv_at_map(22)
            for u, (kc, qh) in enumerate(head_units()):
                scores_unit(0, kc, qh)
                if u < DCH:
                    # Q0h1 + K0h1 projection, chunk u (4 MMs)
                    proj_mm(h0_acc, 0, 0, 1, u)
                    proj_mm(h0_acc, 1, 0, 1, u)
                    if u == DCH - 1:
                        proj_bias(h0_acc, 0, 1)
                if 8 <= u < 22:
                    v_proj(u - 8)
                if u in pv_at:
                    pv_pair(0, pv_at[u])

            # --- head 1 ---
            h1_accA = None
            h1_accB = None
            pv_at = pv_at_map(24)
            for u, (kc, qh) in enumerate(head_units()):
                scores_unit(1, kc, qh)
                if u == 0:
                    v_proj(14)
                elif u == 1:
                    v_proj(15)
                elif u == 2:
                    pv_pair(0, NPAIR - 1)
                elif u == 3:
                    normalize(0, 0, 2)
                elif u == 4:
                    normalize(0, 1, 2)
                elif 8 <= u < 16:
                    c = u - 8
                    if c == 0:
                        h1_accA = proj_acc_alloc("accQK1h0")
                    proj_mm(h1_accA, 0, 1, 0, c)
                    proj_mm(h1_accA, 1, 1, 0, c)
                    if c == DCH - 1:
                        proj_bias(h1_accA, 1, 0)
                elif 16 <= u < 24:
                    c = u - 16
                    if c == 0:
                        h1_accB = proj_acc_alloc("accQK1h1")
                    proj_mm(h1_accB, 0, 1, 1, c)
                    proj_mm(h1_accB, 1, 1, 1, c)
                    if c == DCH - 1:
                        proj_bias(h1_accB, 1, 1)
                if u in pv_at:
                    pv_pair(1, pv_at[u])

            # --- heads 2, 3 ---
            for h in (2, 3):
                pv_pair(h - 1, NPAIR - 1)
                pv_at = pv_at_map(22)
                for u, (kc, qh) in enumerate(head_units()):
                    scores_unit(h, kc, qh)
                    if u == 0:
                        normalize(h - 1, 0, 2)
                    elif u == 1:
                        normalize(h - 1, 1, 2)
                    if u in pv_at:
                        pv_pair(h, pv_at[u])
            pv_pair(HPC - 1, NPAIR - 1)

            # ---- tail: head-3 normalize (quartered) + output projection.
            # PSUM->SBUF movers alternate DVE / ACT (ACT is idle now);
            # output DMA alternates sync / gpsimd queues.
            def out_tb(tb):
                acc = ps.tile([P, 1024], f32, tag="work", name=f"ps_o{tb}")
                for dc in range(2):
                    for ns in range(2):
                        nc.tensor.matmul(
                            acc[:, ns * 512:(ns + 1) * 512],
                            ctx_sb[dc][:, tb * P:(tb + 1) * P],
                            wo_s[:, dc, ns * 512:(ns + 1) * 512],
                            start=(dc == 0), stop=(dc == 1))
                o_sb = sp.tile([P, D], bf16, tag="osb", name=f"osb{tb}")
                if tb % 2 == 0:
                    nc.vector.tensor_copy(o_sb[:], acc[:])
                else:
                    nc.scalar.copy(o_sb[:], acc[:])
                eng = nc.sync if tb % 2 == 0 else nc.gpsimd
                eng.dma_start(out_d[tb * P:(tb + 1) * P, :], o_sb[:])

            for qq in range(4):
                normalize(HPC - 1, qq, 4)
                for tb in range(4 * qq, 4 * qq + 4):
                    out_tb(tb)

    nc.compile()
    return nc


def _get_compiled():
    if "nc" not in _CACHE:
        _CACHE["nc"] = _build()
    return _CACHE["nc"]


def kernel(x, Wq, bq, Wk, bk, Wv, bv, Wo, bo):
    from concourse.bass_utils import run_bass_kernel_spmd

    nc = _get_compiled()
    x = np.asarray(x, dtype=np.float32)
    Wq, bq = np.asarray(Wq, np.float32), np.asarray(bq, np.float32)
    Wk, bk = np.asarray(Wk, np.float32), np.asarray(bk, np.float32)
    Wv, bv = np.asarray(Wv, np.float32), np.asarray(bv, np.float32)
    Wo, bo = np.asarray(Wo, np.float32), np.asarray(bo, np.float32)

    bf = ml_dtypes.bfloat16
    in_maps = []
    for c in range(NCORES):
        b, g = c // 4, c % 4
        cols = slice(g * DL, (g + 1) * DL)
        bq_l, bk_l = bq[cols], bk[cols]
        bqk = np.stack(
            [bq_l[0:P], bq_l[P:2 * P], bk_l[0:P], bk_l[P:2 * P]], axis=1)
        in_maps.append({
            "xt": np.ascontiguousarray(x[b].T).astype(bf),
            "wq": Wq[:, cols].astype(bf),
            "wk": Wk[:, cols].astype(bf),
            "wv": Wv[:, cols].astype(bf),
            "wo": Wo[cols, :].astype(bf),
            "bqk": np.ascontiguousarray(bqk, np.float32),
        })

    _CACHE["in_maps"] = in_maps
    res = run_bass_kernel_spmd(nc, in_maps, list(range(NCORES)))

    # constant row: bv @ Wo + bo (softmax rows sum to 1)
    const_row = bv.astype(np.float64) @ Wo.astype(np.float64) + bo
    out = np.zeros((B, S, D), np.float64)
    for c in range(NCORES):
        out[c // 4] += res.results[c]["out"].astype(np.float64)
    out += const_row
    return out.astype(np.float32)


# revision 10
# speedup vs baseline: 1.0770x; 1.0770x over previous
"""Multi-head attention (B=2, S=2048, D=1024, H=16) on 8 trn2 NeuronCores.

Sharding: core c -> batch b = c // 4, head group g = c % 4 (heads 4g..4g+3).
Each core computes, for its batch shard and 4 heads:
  QT/KT = (x W + b)^T in [d_local, seq] layout, V in [seq, d_local] layout,
  transposed scores S^T[k, q] = K Q^T (so softmax needs no transposes),
  exp via ACT (scale fused), fp8e4m3 DoubleRow PV with an appended ones
  column yielding both the unnormalized context and the softmax row sums,
  normalization via a gpsimd partition-broadcast reciprocal multiply, and
  a partial output projection against a row shard of Wo.
Host sums the 4 partials per batch and adds the constant row bv @ Wo + bo.

Schedule: the ACT engine (exp of 16.8M scores, ~1.11us per [128,1024]
unit) paces the kernel, and engines synchronize on cumulative completion
counters, so every PE instruction emitted between two scores units delays
the second unit's exp by its full duration. Hence:
  - per-head units are (token-half qh, key-chunk kc), scores first in
    each unit, fillers quantized to <=430ns pieces (half V projections,
    one projection chunk, 1-2 PV DoubleRow matmuls) so per-unit PE time
    stays at or under the ACT cadence;
  - PSUM: 4 banks of rotating [128,1024] scores/work tiles, 2 banks for
    the PV context accumulator (one token-half at a time - the exps of a
    whole head stay resident in fp8, bufs=16), and 2 banks for a
    dedicated projection-accumulator pool so the second-half/dblk1
    projections never wait on a normalize of the previous head;
  - PV for token-half 0 runs inside the head (pairs all ready by unit
    16); PV for half 1 + its normalize slide into the next head's first
    units; head 3's half-0 output blocks run inside head 3, so only 8
    output blocks + half-1 normalize remain after the last exp;
  - PSUM->SBUF output movers run on DVE in-head and alternate DVE/ACT in
    the tail; output DMA alternates the sync/gpsimd queues.
"""

import sys

sys.path.insert(0, "/opt/trn_rl_repo")

import numpy as np
import ml_dtypes

B = 2
S = 2048
D = 1024
H = 16
HD = 64
NCORES = 8
HPC = 4          # heads per core
DL = HPC * HD    # 256 local head dims per core
P = 128
KCH = S // P     # 16 key chunks
DCH = D // P     # 8 contraction chunks
TBLK = S // P    # 16 token blocks
NPAIR = KCH // 2
SCALE = 1.0 / np.sqrt(HD)

USE_FP8_PV = True

_CACHE = {}


def _build():
    import concourse.bass as bass  # noqa: F401
    import concourse.mybir as mybir
    import concourse.tile as tile
    from concourse import bacc

    bf16 = mybir.dt.bfloat16
    f32 = mybir.dt.float32
    fp8 = mybir.dt.float8e4
    DR = mybir.MatmulPerfMode.DoubleRow
    Exp = mybir.ActivationFunctionType.Exp

    nc = bacc.Bacc("TRN2", target_bir_lowering=False, debug=False,
                   num_devices=NCORES)

    xT_d = nc.dram_tensor("xt", [D, S], bf16, kind="ExternalInput")
    wq_d = nc.dram_tensor("wq", [D, DL], bf16, kind="ExternalInput")
    wk_d = nc.dram_tensor("wk", [D, DL], bf16, kind="ExternalInput")
    wv_d = nc.dram_tensor("wv", [D, DL], bf16, kind="ExternalInput")
    wo_d = nc.dram_tensor("wo", [DL, D], bf16, kind="ExternalInput")
    bqk_d = nc.dram_tensor("bqk", [P, 4], f32, kind="ExternalInput")
    out_d = nc.dram_tensor("out", [S, D], bf16, kind="ExternalOutput")

    att_dt = fp8 if USE_FP8_PV else bf16

    with tile.TileContext(nc) as tc:
        with (
            tc.tile_pool(name="persist", bufs=1) as pp,
            tc.tile_pool(name="stream", bufs=3) as sp,
            tc.tile_pool(name="psum", bufs=2, space="PSUM") as ps,
        ):
            # ---- input DMAs: weights first (small), then x token-halves.
            bqk_s = pp.tile([P, 4], f32, tag="bqk", name="bqk_s")
            nc.sync.dma_start(bqk_s[:], bqk_d[:])
            wq_s = pp.tile([P, DCH, DL], bf16, tag="wq", name="wq_s")
            wk_s = pp.tile([P, DCH, DL], bf16, tag="wk", name="wk_s")
            wv_s = pp.tile([P, DCH, DL], bf16, tag="wv", name="wv_s")
            xts = [pp.tile([P, S], bf16, tag=f"xt{c}", name=f"xt{c}")
                   for c in range(DCH)]
            for c in range(DCH):
                nc.gpsimd.dma_start(wq_s[:, c, :], wq_d[c * P:(c + 1) * P, :])
            for c in range(DCH):
                nc.sync.dma_start(wk_s[:, c, :], wk_d[c * P:(c + 1) * P, :])
            for c in range(DCH):
                eng = nc.sync if c % 2 == 0 else nc.gpsimd
                eng.dma_start(xts[c][:, 0:1024], xT_d[c * P:(c + 1) * P, 0:1024])
            for c in range(DCH):
                nc.gpsimd.dma_start(wv_s[:, c, :], wv_d[c * P:(c + 1) * P, :])
            for c in range(DCH):
                eng = nc.sync if c % 2 == 1 else nc.gpsimd
                eng.dma_start(xts[c][:, 1024:2048],
                              xT_d[c * P:(c + 1) * P, 1024:2048])
            wo_s = pp.tile([P, 2, D], bf16, tag="wo", name="wo_s")
            for dc in range(2):
                nc.gpsimd.dma_start(wo_s[:, dc, :], wo_d[dc * P:(dc + 1) * P, :])

            qt = [pp.tile([P, S], bf16, tag=f"qt{d}", name=f"qt{d}")
                  for d in range(2)]
            kt = [pp.tile([P, S], bf16, tag=f"kt{d}", name=f"kt{d}")
                  for d in range(2)]

            # ---- phase B: Q/K dblk0 first-token-half projections,
            # consuming each x half-chunk once on arrival.
            qacc = ps.tile([P, 1024], f32, tag="work", name="qacc_h0")
            kacc = ps.tile([P, 1024], f32, tag="work", name="kacc_h0")
            for c in range(DCH):
                for ns in range(2):
                    nc.tensor.matmul(
                        qacc[:, ns * 512:(ns + 1) * 512],
                        wq_s[:, c, 0:P],
                        xts[c][:, ns * 512:(ns + 1) * 512],
                        start=(c == 0), stop=(c == DCH - 1))
                for ns in range(2):
                    nc.tensor.matmul(
                        kacc[:, ns * 512:(ns + 1) * 512],
                        wk_s[:, c, 0:P],
                        xts[c][:, ns * 512:(ns + 1) * 512],
                        start=(c == 0), stop=(c == DCH - 1))
            nc.vector.tensor_scalar_add(qt[0][:, 0:1024], qacc[:],
                                        bqk_s[:, 0:1])
            nc.vector.tensor_scalar_add(kt[0][:, 0:1024], kacc[:],
                                        bqk_s[:, 2:3])

            # ---- projection-accumulator pool: its own 2 PSUM banks, one
            # proj-half at a time, 1 chunk (2 MMs) per unit.
            pj_ref = [None]

            def proj_step(which, dblk, half, c):
                w_s = wq_s if which == 0 else wk_s
                if c == 0:
                    pj_ref[0] = ps.tile([P, 1024], f32, tag="pacc", bufs=1,
                                        name=f"pacc{which}{dblk}{half}")
                acc = pj_ref[0]
                for ns in range(2):
                    nc.tensor.matmul(
                        acc[:, ns * 512:(ns + 1) * 512],
                        w_s[:, c, dblk * P:(dblk + 1) * P],
                        xts[c][:, half * 1024 + ns * 512:
                               half * 1024 + (ns + 1) * 512],
                        start=(c == 0), stop=(c == DCH - 1))
                if c == DCH - 1:
                    t_sb = qt[dblk] if which == 0 else kt[dblk]
                    bcol = dblk if which == 0 else 2 + dblk
                    nc.vector.tensor_scalar_add(
                        t_sb[:, half * 1024:(half + 1) * 1024],
                        acc[:], bqk_s[:, bcol:bcol + 1])

            # V tiles: k-chunk PAIRS [128, 2, 4 heads * 68]; col 68h+64 is
            # the softmax-sum ones column. Emitted as two 4-chunk quanta.
            vts = [None] * NPAIR
            v_ref = [None]

            def v_step(tb, quantum):
                pr, j = tb // 2, tb % 2
                if quantum == 0:
                    if j == 0:
                        vt = pp.tile([P, 2, HPC * 68], att_dt, tag=f"v{pr}",
                                     name=f"v{pr}")
                        v4 = vt.rearrange("p j (h e) -> p j h e", e=68)
                        nc.gpsimd.memset(v4[:, :, :, 64:65], 1.0)
                        vts[pr] = vt
                    v_ref[0] = ps.tile([P, 1024], f32, tag="work",
                                       name=f"ps_v{tb}")
                acc = v_ref[0]
                for kc in range(4 * quantum, 4 * quantum + 4):
                    nc.tensor.matmul(
                        acc[:, 0:DL],
                        xts[kc][:, tb * P:(tb + 1) * P],
                        wv_s[:, kc, :],
                        start=(kc == 0), stop=(kc == DCH - 1))
                if quantum == 1:
                    v4 = vts[pr].rearrange("p j (h e) -> p j h e", e=68)
                    nc.vector.tensor_copy(
                        v4[:, j, :, 0:64],
                        acc[:, 0:DL].rearrange("p (h e) -> p h e", e=64))

            etps = [None] * NPAIR
            ctx_ps = {}      # qh -> current [P,1024] ctx psum tile
            ctx_sb = [pp.tile([P, S], bf16, tag=f"ctx{dc}", name=f"ctx{dc}")
                      for dc in range(2)]

            def scores_unit(h, kc, qh):
                dblk = h // 2
                roff = 64 * (h % 2)
                pr, j = kc // 2, kc % 2
                if j == 0 and qh == 0:
                    # whole head resident: 16 buffers so PV of half 1 can
                    # trail into the next head without stalling allocs.
                    etps[pr] = sp.tile([P, 2, S], att_dt, tag="expt",
                                       bufs=16, name=f"expt{h}_{pr}")
                sc = ps.tile([P, 1024], f32, tag="work",
                             name=f"sc{h}_{kc}_{qh}")
                for ns in range(2):
                    nc.tensor.matmul(
                        sc[:, ns * 512:(ns + 1) * 512],
                        kt[dblk][roff:roff + 64, kc * P:(kc + 1) * P],
                        qt[dblk][roff:roff + 64,
                                 qh * 1024 + ns * 512:qh * 1024 + (ns + 1) * 512],
                        start=True, stop=True)
                nc.scalar.activation(
                    etps[pr][:, j, qh * 1024:(qh + 1) * 1024], sc[:],
                    Exp, scale=float(SCALE))

            def pv_step(h, qh, pr, ets=None):
                """One pair's PV for one token-half: 2 fp8 DR matmuls."""
                if pr == 0:
                    ctx_ps[qh] = ps.tile([P, 1024], f32, tag="ctx", bufs=1,
                                         name=f"ps_ctx{h}_{qh}")
                cps = ctx_ps[qh]
                v4 = vts[pr].rearrange("p j (h e) -> p j h e", e=68)
                et = ets if ets is not None else etps[pr]
                for ns in range(2):
                    nc.tensor.matmul(
                        cps[0:65, ns * 512:(ns + 1) * 512],
                        v4[:, :, h, 0:65],
                        et[:, :, qh * 1024 + ns * 512:
                           qh * 1024 + (ns + 1) * 512],
                        start=(pr == 0), stop=(pr == NPAIR - 1),
                        perf_mode=DR)

            def normalize(h, qh, part, nparts=2):
                """Normalize 1/nparts of head h's half-qh context."""
                dblk = h // 2
                roff = 64 * (h % 2)
                w = 1024 // nparts
                cps = ctx_ps[qh]
                hs = slice(part * w, (part + 1) * w)
                gs = slice(qh * 1024 + part * w, qh * 1024 + (part + 1) * w)
                srow = sp.tile([1, w], f32, tag=f"srow{w}", bufs=2,
                               name=f"srow{h}_{qh}_{part}")
                nc.vector.tensor_copy(srow[:], cps[64:65, hs])
                rec = sp.tile([1, w], f32, tag=f"rec{w}", bufs=2,
                              name=f"rec{h}_{qh}_{part}")
                nc.vector.reciprocal_approx_fast(rec[:], srow[:])
                bc = sp.tile([64, w], f32, tag=f"bc{w}", bufs=2,
                             name=f"bc{h}_{qh}_{part}")
                nc.gpsimd.partition_broadcast(bc[:], rec[:])
                nc.vector.tensor_mul(
                    ctx_sb[dblk][roff:roff + 64, gs],
                    cps[0:64, hs], bc[:])

            def out_tb(tb, mover):
                acc = ps.tile([P, 1024], f32, tag="work", name=f"ps_o{tb}")
                for dc in range(2):
                    for ns in range(2):
                        nc.tensor.matmul(
                            acc[:, ns * 512:(ns + 1) * 512],
                            ctx_sb[dc][:, tb * P:(tb + 1) * P],
                            wo_s[:, dc, ns * 512:(ns + 1) * 512],
                            start=(dc == 0), stop=(dc == 1))
                o_sb = sp.tile([P, D], bf16, tag="osb", name=f"osb{tb}")
                if mover == 0:
                    nc.vector.tensor_copy(o_sb[:], acc[:])
                else:
                    nc.scalar.copy(o_sb[:], acc[:])
                eng = nc.sync if tb % 2 == 0 else nc.gpsimd
                eng.dma_start(out_d[tb * P:(tb + 1) * P, :], o_sb[:])

            # ---- heads loop -------------------------------------------
            # 32 units per head (qh0 kc0-15, then qh1 kc0-15), scores
            # first in each unit, fillers quantized to <=430ns. PV of a
            # head's half-1 + its normalize trail into the next head.

            def emit_head0():
                for u in range(32):
                    qh, kc = (0, u) if u < 16 else (1, u - 16)
                    scores_unit(0, kc, qh)
                    if u < 8:
                        proj_step(1, 0, 1, u)        # K0 second-half
                    elif u < 16:
                        proj_step(0, 0, 1, u - 8)    # Q0 second-half
                    v_step(u // 2, u % 2)            # V tb 0-15
                    if u >= 25:
                        pv_step(0, 0, u - 25)        # pairs 0-6

            def emit_head1(ets_prev):
                for u in range(32):
                    qh, kc = (0, u) if u < 16 else (1, u - 16)
                    scores_unit(1, kc, qh)
                    if u < 8:
                        proj_step(0, 1, 0, u)        # Q1 first-half
                    elif u < 16:
                        proj_step(1, 1, 0, u - 8)    # K1 first-half
                    elif u < 24:
                        proj_step(1, 1, 1, u - 16)   # K1 second-half
                    else:
                        proj_step(0, 1, 1, u - 24)   # Q1 second-half
                    if u == 0:
                        pv_step(0, 0, NPAIR - 1)     # needs V tb15 copy
                    elif u == 1:
                        normalize(0, 0, 0, 2)
                    elif u == 2:
                        normalize(0, 0, 1, 2)
                    if 6 <= u < 14:
                        pv_step(0, 1, u - 6, ets=ets_prev[u - 6])
                    if u == 14:
                        normalize(0, 1, 0, 2)
                    elif u == 16:
                        normalize(0, 1, 1, 2)
                    if 18 <= u < 26:
                        pv_step(1, 0, u - 18)
                    if u == 27:
                        normalize(1, 0, 0, 2)
                    elif u == 29:
                        normalize(1, 0, 1, 2)

            def emit_head23(h, ets_prev):
                for u in range(32):
                    qh, kc = (0, u) if u < 16 else (1, u - 16)
                    scores_unit(h, kc, qh)
                    if 5 <= u < 13:
                        pv_step(h - 1, 1, u - 5, ets=ets_prev[u - 5])
                    if u == 13:
                        normalize(h - 1, 1, 0, 2)
                    elif u == 15:
                        normalize(h - 1, 1, 1, 2)
                    if 18 <= u < 26:
                        pv_step(h, 0, u - 18)
                    if h < 3:
                        if u == 27:
                            normalize(h, 0, 0, 2)
                        elif u == 29:
                            normalize(h, 0, 1, 2)
                    else:
                        if 26 <= u < 30:
                            normalize(3, 0, u - 26, 4)
                        elif u == 30:
                            out_tb(0, 0)
                        elif u == 31:
                            out_tb(1, 0)

            emit_head0()
            ets0 = list(etps)
            emit_head1(ets0)
            ets1 = list(etps)
            emit_head23(2, ets1)
            ets2 = list(etps)
            emit_head23(3, ets2)
            ets3 = list(etps)

            # ---- tail: head-3 half-1 PV + normalize, out blocks 2-15.
            # First the half-0 output blocks interleave with the PV MMs;
            # movers split between DVE (0) and the now-idle ACT (1).
            tail_plan = [
                ("pv", 0), ("pv", 1), ("out", 2, 0), ("pv", 2),
                ("out", 3, 1), ("pv", 3), ("out", 4, 0), ("pv", 4),
                ("out", 5, 1), ("pv", 5), ("out", 6, 0), ("pv", 6),
                ("out", 7, 1), ("pv", 7),
            ]
            for step in tail_plan:
                if step[0] == "pv":
                    pv_step(3, 1, step[1], ets=ets3[step[1]])
                else:
                    out_tb(step[1], step[2])
            normalize(3, 1, 0, 4)
            normalize(3, 1, 1, 4)
            out_tb(8, 0)
            out_tb(9, 1)
            normalize(3, 1, 2, 4)
            out_tb(10, 0)
            out_tb(11, 1)
            normalize(3, 1, 3, 4)
            out_tb(12, 0)
            out_tb(13, 1)
            out_tb(14, 0)
            out_tb(15, 1)

    nc.compile()
    return nc


def _get_compiled():
    if "nc" not in _CACHE:
        _CACHE["nc"] = _build()
    return _CACHE["nc"]


def kernel(x, Wq, bq, Wk, bk, Wv, bv, Wo, bo):
    from concourse.bass_utils import run_bass_kernel_spmd

    nc = _get_compiled()
    x = np.asarray(x, dtype=np.float32)
    Wq, bq = np.asarray(Wq, np.float32), np.asarray(bq, np.float32)
    Wk, bk = np.asarray(Wk, np.float32), np.asarray(bk, np.float32)
    Wv, bv = np.asarray(Wv, np.float32), np.asarray(bv, np.float32)
    Wo, bo = np.asarray(Wo, np.float32), np.asarray(bo, np.float32)

    bf = ml_dtypes.bfloat16
    in_maps = []
    for c in range(NCORES):
        b, g = c // 4, c % 4
        cols = slice(g * DL, (g + 1) * DL)
        bq_l, bk_l = bq[cols], bk[cols]
        bqk = np.stack(
            [bq_l[0:P], bq_l[P:2 * P], bk_l[0:P], bk_l[P:2 * P]], axis=1)
        in_maps.append({
            "xt": np.ascontiguousarray(x[b].T).astype(bf),
            "wq": Wq[:, cols].astype(bf),
            "wk": Wk[:, cols].astype(bf),
            "wv": Wv[:, cols].astype(bf),
            "wo": Wo[cols, :].astype(bf),
            "bqk": np.ascontiguousarray(bqk, np.float32),
        })

    _CACHE["in_maps"] = in_maps
    res = run_bass_kernel_spmd(nc, in_maps, list(range(NCORES)))

    const_row = bv.astype(np.float64) @ Wo.astype(np.float64) + bo
    out = np.zeros((B, S, D), np.float64)
    for c in range(NCORES):
        out[c // 4] += res.results[c]["out"].astype(np.float64)
    out += const_row
    return out.astype(np.float32)
